# revision 67
# baseline (speedup 1.0000x reference)
"""Trainium2 Bass kernel for top-2 MoE routing (B=4, S=2048, D=1024, E=8, K=2).

Strategy: data-parallel over tokens across 8 NeuronCores (1024 tokens/core),
expert/gate weights replicated. Per core, fully on-device:
  1. gate scores in ~fp32 precision via bf16 hi/res split matmuls; all 8
     experts' bias rows are partition-broadcast up front while PE is idle
  2. top-2 selection + softmax weights; slot bases flattened via PE
     selector matmuls (never chain small DMAs - each link costs ~1.5-2us)
  3. slot-order (tokid, w) records via ONE batched dma_scatter_add into a
     [CAP, 64] f32 table (slots unique -> no CCE add-collision races);
     int16 idx tables built with permuted-identity PE transposes + a
     matmul-based 16->128 partition replication; expert-0's idx slice is
     read back first so its gather fires early
  4. per-expert TRANSPOSED dma_gather (xgT straight from DRAM, no PE
     transposes), dense matmul per slot tile; bias and softmax weight are
     folded into the PSUM evacuation, so ydram rows hold w*(x@We + be)
  5. combine: token-quarter-chunked batched dma_gathers of each token's
     two expert rows from ydram + a single add per tile, bf16 output
"""

import numpy as np
import ml_dtypes

import concourse.bacc as bacc
import concourse.mybir as mybir
import concourse.tile as tile
from concourse.bass_utils import run_bass_kernel_spmd

BF16 = ml_dtypes.bfloat16
P = 128          # partitions
D = 1024         # model dim
E = 8            # experts
TOK = 1024       # tokens per core
NT = TOK // P    # token tiles per core
C = 384          # slot capacity per expert (max observed load 294)
ST = C // P      # slot tiles per expert
NS = E * ST      # total slot tiles
CAP = E * C      # total slots
NREC = 2 * TOK   # token-rank records
NCORES = 8

F32 = mybir.dt.float32
BF = mybir.dt.bfloat16
I16 = mybir.dt.int16
AX = mybir.AxisListType.X
OP = mybir.AluOpType
EXP = mybir.ActivationFunctionType.Exp


def sl(i, n):
    return slice(i * n, (i + 1) * n)


def build_nc(timing_reps=0):
    nc = bacc.Bacc("TRN2", target_bir_lowering=False, debug=False)
    ki = "Internal" if timing_reps else "ExternalInput"
    ko = "Internal" if timing_reps else "ExternalOutput"

    xh = nc.dram_tensor("xh", [TOK, D], BF, kind=ki)
    xhT = nc.dram_tensor("xhT", [D, TOK], BF, kind=ki)
    xrT = nc.dram_tensor("xrT", [D, TOK], BF, kind=ki)
    wgb = nc.dram_tensor("wgb", [D, 2 * E], BF, kind=ki)
    bgb = nc.dram_tensor("bgb", [P, NT * E], F32, kind=ki)
    web = nc.dram_tensor("web", [E, D, D], BF, kind=ki)
    beb = nc.dram_tensor("beb", [1, E * D], BF, kind=ki)
    idb = nc.dram_tensor("idb", [P, P], BF, kind=ki)
    idf = nc.dram_tensor("idf", [P, P], F32, kind=ki)
    permf = nc.dram_tensor("permf", [P, P], F32, kind=ki)
    rep16 = nc.dram_tensor("rep16", [16, P], F32, kind=ki)
    u128 = nc.dram_tensor("u128", [P, P], BF, kind=ki)
    u8x = nc.dram_tensor("u8x", [E, E], F32, kind=ki)
    ones1 = nc.dram_tensor("ones1", [1, P], F32, kind=ki)
    ones1b = nc.dram_tensor("ones1b", [1, P], BF, kind=ki)
    onesc = nc.dram_tensor("onesc", [P, 1], BF, kind=ki)
    ecv = nc.dram_tensor("ecv", [P, NT * E], F32, kind=ki)
    tokid = nc.dram_tensor("tokid", [P, NT], F32, kind=ki)
    recdram = nc.dram_tensor("recdram", [CAP, 64], F32, kind="Internal")
    ydram = nc.dram_tensor("ydram", [CAP, D], BF, kind="Internal")
    sdram = nc.dram_tensor("sdram", [P, 16], F32, kind="Internal")
    cdram = nc.dram_tensor("cdram", [2, P, NT], F32, kind="Internal")
    out = nc.dram_tensor("out", [TOK, D], BF, kind=ko)
    if timing_reps:
        tdum = nc.dram_tensor("tdum", [1, 4], F32, kind="ExternalInput")
        outt = nc.dram_tensor("outt", [1, 4], F32, kind="ExternalOutput")

    with tile.TileContext(nc) as tc:
        with (
            tc.tile_pool(name="const", bufs=1) as const,
            tc.tile_pool(name="persist", bufs=1) as persist,
            tc.tile_pool(name="wp", bufs=3) as wp,
            tc.tile_pool(name="gp", bufs=3) as gp,
            tc.tile_pool(name="yp", bufs=4) as yp,
            tc.tile_pool(name="fin", bufs=1) as fin,
            tc.tile_pool(name="small", bufs=2) as small,
            tc.tile_pool(name="ps_s", bufs=2, space="PSUM") as ps_s,
            tc.tile_pool(name="ps_tr", bufs=2, space="PSUM") as ps_tr,
            tc.tile_pool(name="ps_mm", bufs=4, space="PSUM") as ps_mm,
        ):
            for _rep in range(max(1, timing_reps)):
                # ---- early loads: wg first (tiny, gating-critical), then x ----
                wg_sb = const.tile([P, 8, 2 * E], BF)
                nc.sync.dma_start(out=wg_sb[:], in_=wgb[:].rearrange("(c p) e -> p c e", p=P))
                xhT_sb = persist.tile([P, 8, TOK], BF)
                xrT_sb = persist.tile([P, 8, TOK], BF)
                nc.sync.dma_start(out=xhT_sb[:, 0:4, :],
                                  in_=xhT[0:512].rearrange("(c p) t -> p c t", p=P))
                nc.sync.dma_start(out=xrT_sb[:, 0:4, :],
                                  in_=xrT[0:512].rearrange("(c p) t -> p c t", p=P))
                nc.sync.dma_start(out=xhT_sb[:, 4:8, :],
                                  in_=xhT[512:1024].rearrange("(c p) t -> p c t", p=P))
                nc.sync.dma_start(out=xrT_sb[:, 4:8, :],
                                  in_=xrT[512:1024].rearrange("(c p) t -> p c t", p=P))

                idb_sb = const.tile([P, P], BF)
                nc.sync.dma_start(out=idb_sb[:], in_=idb[:])
                idf_sb = const.tile([P, P], F32)
                nc.sync.dma_start(out=idf_sb[:], in_=idf[:])
                permf_sb = const.tile([P, P], F32)
                nc.sync.dma_start(out=permf_sb[:], in_=permf[:])
                rep16_sb = const.tile([16, P], F32)
                nc.sync.dma_start(out=rep16_sb[:], in_=rep16[:])
                u128_sb = const.tile([P, P], BF)
                nc.sync.dma_start(out=u128_sb[:], in_=u128[:])
                u8x_sb = const.tile([E, E], F32)
                nc.sync.dma_start(out=u8x_sb[:], in_=u8x[:])
                ones1_sb = const.tile([1, P], F32)
                nc.sync.dma_start(out=ones1_sb[:], in_=ones1[:])
                ones1b_sb = const.tile([1, P], BF)
                nc.sync.dma_start(out=ones1b_sb[:], in_=ones1b[:])
                onesc_sb = const.tile([P, 1], BF)
                nc.sync.dma_start(out=onesc_sb[:], in_=onesc[:])
                ecv_sb = const.tile([P, NT, E], F32)
                nc.sync.dma_start(out=ecv_sb[:], in_=ecv[:])
                bgb_sb = const.tile([P, NT, E], F32)
                nc.sync.dma_start(out=bgb_sb[:], in_=bgb[:])
                tokid_sb = const.tile([P, NT, 1], F32)
                nc.sync.dma_start(out=tokid_sb[:], in_=tokid[:])
                beb_sb = const.tile([1, E * D], BF)
                nc.sync.dma_start(out=beb_sb[:], in_=beb[:])
                we_tiles = {}

                def load_we(e):
                    wt = wp.tile([P, 8, D], BF, tag="wp", name=f"we{e}")
                    nc.sync.dma_start(
                        out=wt[:, 0:4, :],
                        in_=web[e, 0:512].rearrange("(c p) h -> p c h", p=P))
                    nc.sync.dma_start(
                        out=wt[:, 4:8, :],
                        in_=web[e, 512:1024].rearrange("(c p) h -> p c h", p=P))
                    we_tiles[e] = wt

                load_we(0)
                load_we(1)

                # zero-init record-table cols 0:2 (all that is ever read)
                zrec = const.tile([P, NS, 2], F32)
                nc.vector.memset(zrec[:], 0.0)
                nc.sync.dma_start(
                    out=recdram[:, 0:2].rearrange("(s p) e -> p s e", p=P), in_=zrec[:])

                # record rows: memset + token ids early (w fields come later)
                rec = persist.tile([P, NT, 2, 64], F32)
                nc.vector.memset(rec[:], 0.0)
                nc.any.tensor_copy(out=rec[:, :, 0, 0:1], in_=tokid_sb[:])
                nc.any.tensor_copy(out=rec[:, :, 1, 0:1], in_=tokid_sb[:])

                # persistent routing state
                W_sb = persist.tile([P, NT, E], F32)
                selp_sb = persist.tile([P, NT, E], BF)
                s12f = persist.tile([P, NT, 2], F32)
                cnts_sb = persist.tile([E, NT], F32)
                base_sb = persist.tile([NT, E], F32)

                # ---- phase 1: gating + vectorized top-2 routing ----
                sco_all = small.tile([P, NT, 2 * E], F32)
                for t in range(NT):
                    psg = ps_s.tile([P, 2 * E], F32, tag="pss")
                    k = 0
                    for half in range(2):
                        for src in (xhT_sb, xrT_sb):
                            for c in range(4 * half, 4 * half + 4):
                                nc.tensor.matmul(
                                    psg[:],
                                    lhsT=src[:, c, sl(t, P)],
                                    rhs=wg_sb[:, c, :],
                                    start=(k == 0),
                                    stop=(k == 15),
                                )
                                k += 1
                    nc.any.tensor_copy(out=sco_all[:, t, :], in_=psg[:])

                sca = small.tile([P, NT, E], F32)
                nc.vector.tensor_tensor(out=sca[:], in0=sco_all[:, :, 0:E],
                                        in1=sco_all[:, :, E:2 * E], op=OP.add)
                nc.vector.tensor_tensor(out=sca[:], in0=sca[:], in1=bgb_sb[:], op=OP.add)

                m1 = small.tile([P, NT, 1], F32)
                nc.vector.reduce_max(out=m1[:], in_=sca[:], axis=AX)
                eq1 = small.tile([P, NT, E], F32)
                nc.vector.tensor_tensor(out=eq1[:], in0=sca[:],
                                        in1=m1[:].to_broadcast([P, NT, E]), op=OP.is_equal)
                nc.vector.tensor_scalar(out=eq1[:], in0=eq1[:], scalar1=1e30,
                                        scalar2=None, op0=OP.mult)
                sm2 = small.tile([P, NT, E], F32)
                nc.vector.tensor_tensor(out=sm2[:], in0=sca[:], in1=eq1[:], op=OP.subtract)
                m2 = small.tile([P, NT, 1], F32)
                nc.vector.reduce_max(out=m2[:], in_=sm2[:], axis=AX)
                sel = small.tile([P, NT, E], F32)
                nc.vector.tensor_tensor(out=sel[:], in0=sca[:],
                                        in1=m2[:].to_broadcast([P, NT, E]), op=OP.is_ge)
                dm = small.tile([P, NT, E], F32)
                nc.vector.tensor_tensor(out=dm[:], in0=sca[:],
                                        in1=m1[:].to_broadcast([P, NT, E]), op=OP.subtract)
                u = small.tile([P, NT, E], F32)
                nc.scalar.activation(out=u[:], in_=dm[:], func=EXP)
                uw = small.tile([P, NT, E], F32)
                nc.vector.tensor_tensor(out=uw[:], in0=u[:], in1=sel[:], op=OP.mult)
                den = small.tile([P, NT, 1], F32)
                nc.vector.reduce_sum(out=den[:], in_=uw[:], axis=AX)
                rde = small.tile([P, NT, 1], F32)
                nc.vector.reciprocal(out=rde[:], in_=den[:])
                nc.vector.tensor_tensor(out=W_sb[:], in0=uw[:],
                                        in1=rde[:].to_broadcast([P, NT, E]), op=OP.mult)
                nc.vector.tensor_copy(out=selp_sb[:], in_=sel[:])

                # all experts' bias rows broadcast across partitions now,
                # while the PE is otherwise idle during routing
                brep_all = persist.tile([P, E, D], BF)
                for e in range(E):
                    for h in range(2):
                        psb = ps_mm.tile([P, 512], F32, tag="pmm")
                        nc.tensor.matmul(psb[:], lhsT=ones1b_sb[:],
                                         rhs=beb_sb[0:1, e * D + h * 512:
                                                    e * D + (h + 1) * 512],
                                         start=True, stop=True)
                        nc.any.tensor_copy(out=brep_all[:, e, sl(h, 512)], in_=psb[:])

                # ---- phase 2: prefix sums -> slot ids ----
                # within-tile exclusive prefix for ALL tiles in one matmul
                psp = ps_s.tile([P, NT, E], F32, tag="pss")
                nc.tensor.matmul(psp[:].rearrange("p t e -> p (t e)"),
                                 lhsT=u128_sb[:],
                                 rhs=selp_sb[:].rearrange("p t e -> p (t e)"),
                                 start=True, stop=True)
                slotf_sb = persist.tile([P, NT, E], F32)
                nc.vector.tensor_tensor(out=slotf_sb[:], in0=psp[:], in1=selp_sb[:],
                                        op=OP.subtract)
                for t in range(NT):
                    psc = ps_s.tile([E, 1], F32, tag="pss")
                    nc.tensor.matmul(psc[:], lhsT=selp_sb[:, t, :], rhs=onesc_sb[:],
                                     start=True, stop=True)
                    nc.any.tensor_copy(out=cnts_sb[:, t:t + 1], in_=psc[:])

                pst = ps_s.tile([NT, E], F32, tag="pss")
                nc.tensor.transpose(out=pst[:], in_=cnts_sb[:], identity=idf_sb[0:E, 0:E])
                cntT = small.tile([NT, E], F32)
                nc.any.tensor_copy(out=cntT[:], in_=pst[:])
                psb = ps_s.tile([NT, E], F32, tag="pss")
                nc.tensor.matmul(psb[:], lhsT=u8x_sb[:], rhs=cntT[:], start=True, stop=True)
                nc.any.tensor_copy(out=base_sb[:], in_=psb[:])
                # flatten base [NT, E] -> [1, NT*E] with selector matmuls
                # (PE is idle here; replaces 8 serialized tiny DMAs)
                bfp = ps_tr.tile([1, NT * E], F32, tag="ptr")
                for t in range(NT):
                    nc.tensor.matmul(bfp[0:1, sl(t, E)],
                                     lhsT=idf_sb[0:NT, t:t + 1], rhs=base_sb[:],
                                     start=True, stop=True)
                base_flat = persist.tile([1, NT * E], F32)
                nc.any.tensor_copy(out=base_flat[:], in_=bfp[:])
                bball_ps = ps_s.tile([P, NT * E], F32, tag="pss")
                nc.tensor.matmul(bball_ps[:], lhsT=ones1_sb[:], rhs=base_flat[:],
                                 start=True, stop=True)

                slm = small.tile([P, NT, E], F32)
                nc.vector.tensor_scalar(out=slm[:], in0=selp_sb[:], scalar1=-1e6,
                                        scalar2=1e6, op0=OP.mult, op1=OP.add)
                nc.vector.tensor_tensor(out=slm[:], in0=slm[:], in1=slotf_sb[:], op=OP.add)
                nc.vector.tensor_tensor(out=slm[:], in0=slm[:],
                                        in1=bball_ps[:].rearrange("p (t e) -> p t e", e=E),
                                        op=OP.add)
                nc.vector.tensor_tensor(out=slm[:], in0=slm[:], in1=ecv_sb[:], op=OP.add)
                s1v = s12f[:, :, 0:1]
                nc.vector.tensor_reduce(out=s1v, in_=slm[:], axis=AX, op=OP.min)
                eqs = small.tile([P, NT, E], F32)
                nc.vector.tensor_tensor(out=eqs[:], in0=slm[:],
                                        in1=s1v.to_broadcast([P, NT, E]), op=OP.is_equal)
                nc.vector.tensor_scalar(out=eqs[:], in0=eqs[:], scalar1=1e6,
                                        scalar2=None, op0=OP.mult)
                slm2 = small.tile([P, NT, E], F32)
                nc.vector.tensor_tensor(out=slm2[:], in0=slm[:], in1=eqs[:], op=OP.add)
                nc.vector.tensor_reduce(out=s12f[:, :, 1:2], in_=slm2[:], axis=AX,
                                        op=OP.min)

                # rank-0 weight per token; rank-1 weight = 1 - w1
                eqm1 = small.tile([P, NT, E], F32)
                nc.vector.tensor_tensor(out=eqm1[:], in0=slm[:],
                                        in1=s1v.to_broadcast([P, NT, E]), op=OP.is_equal)
                nc.vector.tensor_tensor(out=eqm1[:], in0=eqm1[:], in1=W_sb[:], op=OP.mult)
                w1 = small.tile([P, NT, 1], F32)
                nc.vector.reduce_sum(out=w1[:], in_=eqm1[:], axis=AX)

                # ---- phase 3: slot-order records via one dma_scatter_add ----
                # record row j=(t*2+r)*128+p -> (tokid, w_r); scattered to slot
                nc.any.tensor_copy(out=rec[:, :, 0, 1:2], in_=w1[:])
                nc.vector.tensor_scalar(out=rec[:, :, 1, 1:2], in0=w1[:], scalar1=-1.0,
                                        scalar2=1.0, op0=OP.mult, op1=OP.add)

                # record-scatter idxs: value s12f[p,t,r] at [j%16, j//16],
                # j=(t*2+r)*128+p; built with two PE transposes + a partition
                # fold, then replicated across the 8 16-partition stripes.
                pt1 = ps_tr.tile([16, P], F32, tag="ptr")
                nc.tensor.transpose(out=pt1[:], in_=s12f[:].rearrange("p t r -> p (t r)"),
                                    identity=permf_sb[:])
                t1c = small.tile([16, P], F32)
                nc.any.tensor_copy(out=t1c[:], in_=pt1[:])
                pt2 = ps_tr.tile([P, 16], F32, tag="ptr")
                nc.tensor.transpose(out=pt2[:], in_=t1c[:],
                                    identity=idf_sb[0:16, 0:16])
                t2c = small.tile([P, 16], F32)
                nc.any.tensor_copy(out=t2c[:], in_=pt2[:])
                # t2c partition (q*8+rr), free m -> recidx[q, m*8+rr]:
                # DRAM hop + fold read + DVE permute + matmul stripe-replicate
                nc.sync.dma_start(out=sdram[:], in_=t2c[:])
                s2 = small.tile([16, NREC // 16], F32)
                nc.sync.dma_start(
                    out=s2[:], in_=sdram[:].rearrange("(q rr) m -> q (rr m)", rr=8))
                rf = small.tile([16, NREC // 16], F32)
                nc.any.tensor_copy(
                    out=rf[:].rearrange("q (m rr) -> q m rr", rr=8),
                    in_=s2[:].rearrange("q (rr m) -> q m rr", m=16))
                psri = ps_tr.tile([P, NREC // 16], F32, tag="ptr")
                nc.tensor.matmul(psri[:], lhsT=rep16_sb[:], rhs=rf[:],
                                 start=True, stop=True)
                recidx = persist.tile([P, NREC // 16], I16)
                nc.any.tensor_copy(out=recidx[:], in_=psri[:])

                nc.gpsimd.dma_scatter_add(
                    out_ap=recdram[:],
                    in_ap=rec[:].rearrange("p t r e -> p (t r) e"),
                    idxs_ap=recidx[:],
                    num_idxs=NREC, num_idxs_reg=NREC, elem_size=64,
                )

                # readbacks: expert-0's gather-idx slice first so its big
                # gather fires ~2.5us sooner; the rest follows in parallel
                gthidx3 = persist.tile([P, CAP // 16], I16)
                NE0 = C // 16
                gf0 = small.tile([16, NE0, 1], F32, tag="gf0", name="gf0")
                nc.sync.dma_start(
                    out=gf0[:],
                    in_=recdram[0:C, 0:1].rearrange("(m q) e -> q m e", q=16))
                psg0 = ps_tr.tile([P, NE0], F32, tag="ptr")
                nc.tensor.matmul(psg0[:], lhsT=rep16_sb[:], rhs=gf0[:, :, 0],
                                 start=True, stop=True)
                nc.any.tensor_copy(out=gthidx3[:, 0:NE0], in_=psg0[:])
                gf = small.tile([16, CAP // 16 - NE0, 1], F32)
                nc.sync.dma_start(
                    out=gf[:],
                    in_=recdram[C:CAP, 0:1].rearrange("(m q) e -> q m e", q=16))
                psgi = ps_tr.tile([P, CAP // 16 - NE0], F32, tag="ptr")
                nc.tensor.matmul(psgi[:], lhsT=rep16_sb[:], rhs=gf[:, :, 0],
                                 start=True, stop=True)
                nc.any.tensor_copy(out=gthidx3[:, NE0:], in_=psgi[:])
                gthidx = gthidx3[:, :]
                wslot = persist.tile([P, NS, 1], F32)
                nc.sync.dma_start(
                    out=wslot[:],
                    in_=recdram[:, 1:2].rearrange("(c p) e -> p c e", p=P))

                # ---- phase 4: per-expert batched gather + dense matmuls ----
                def expert_body(e):
                    if e + 2 < E:
                        load_we(e + 2)
                    we_t = we_tiles[e]
                    # transposed gather: xgT[d%128, d//128, slot] straight from DRAM
                    xgT = gp.tile([P, 8, C], BF)
                    nc.gpsimd.dma_gather(
                        out_ap=xgT[:], in_ap=xh[:],
                        idxs_ap=gthidx[:, sl(e, C // 16)],
                        num_idxs=C, num_idxs_reg=C, elem_size=D, transpose=True,
                    )
                    for s in range(ST):
                        ysb = yp.tile([P, D], BF)
                        for h in range(2):
                            psy = ps_mm.tile([P, 512], F32, tag="pmm")
                            for c in range(8):
                                nc.tensor.matmul(psy[:], lhsT=xgT[:, c, sl(s, P)],
                                                 rhs=we_t[:, c, sl(h, 512)],
                                                 start=(c == 0), stop=(c == 7))
                            # ysb = (x@We + be) * w  -> bias rides into ydram
                            nc.any.tensor_tensor(out=ysb[:, sl(h, 512)], in0=psy[:],
                                                 in1=brep_all[:, e, sl(h, 512)],
                                                 op=OP.add)
                            nc.any.tensor_scalar(out=ysb[:, sl(h, 512)],
                                                 in0=ysb[:, sl(h, 512)],
                                                 scalar1=wslot[:, e * ST + s, 0:1],
                                                 scalar2=None, op0=OP.mult)
                        nc.sync.dma_start(out=ydram[sl(e * ST + s, P), :], in_=ysb[:])

                # combine idxs: token -> slot of rank r, int16 wrapped layout
                cidx = []
                for r in range(2):
                    pa = ps_tr.tile([NT, P], F32, tag="ptr")
                    nc.tensor.transpose(out=pa[:], in_=s12f[:, :, r], identity=permf_sb[:])
                    ac = small.tile([NT, P], F32)
                    nc.any.tensor_copy(out=ac[:], in_=pa[:])
                    pb = ps_tr.tile([P, NT], F32, tag="ptr")
                    nc.tensor.transpose(out=pb[:], in_=ac[:],
                                        identity=idf_sb[0:NT, 0:NT])
                    bc = small.tile([P, NT], F32)
                    nc.any.tensor_copy(out=bc[:], in_=pb[:])
                    nc.sync.dma_start(out=cdram[r], in_=bc[:])
                    c2 = small.tile([16, TOK // 16], F32, tag=f"c2_{r}", name=f"c2_{r}")
                    nc.sync.dma_start(
                        out=c2[:], in_=cdram[r].rearrange("(q rr) t -> q (rr t)", rr=8))
                    cf = small.tile([16, TOK // 16], F32, tag=f"cf_{r}", name=f"cf_{r}")
                    nc.any.tensor_copy(
                        out=cf[:].rearrange("q (t rr) -> q t rr", rr=8),
                        in_=c2[:].rearrange("q (rr t) -> q t rr", t=NT))
                    psci = ps_tr.tile([P, TOK // 16], F32, tag="ptr")
                    nc.tensor.matmul(psci[:], lhsT=rep16_sb[:], rhs=cf[:],
                                     start=True, stop=True)
                    cx = persist.tile([P, TOK // 16], I16, tag=f"cidx{r}", name=f"cidx{r}")
                    nc.any.tensor_copy(out=cx[:], in_=psci[:])
                    cidx.append(cx)

                for e in range(E):
                    expert_body(e)

                # ---- phase 5: combine the two expert rows (bias already in
                # ydram); token-half chunks so adds overlap the 2nd gather ----
                g1 = fin.tile([P, NT, D], BF, tag="g1", name="g1")
                g2 = fin.tile([P, NT, D], BF, tag="g2", name="g2")
                HT = TOK // 4
                for hh in range(4):
                    ts0 = hh * (NT // 4)
                    nc.gpsimd.dma_gather(
                        out_ap=g1[:, ts0:ts0 + NT // 4, :], in_ap=ydram[:],
                        idxs_ap=cidx[0][:, sl(hh, HT // 16)],
                        num_idxs=HT, num_idxs_reg=HT, elem_size=D, transpose=False,
                    )
                    nc.gpsimd.dma_gather(
                        out_ap=g2[:, ts0:ts0 + NT // 4, :], in_ap=ydram[:],
                        idxs_ap=cidx[1][:, sl(hh, HT // 16)],
                        num_idxs=HT, num_idxs_reg=HT, elem_size=D, transpose=False,
                    )
                    for t in range(ts0, ts0 + NT // 4):
                        acc = yp.tile([P, D], BF, tag="acc", name="acc")
                        nc.any.tensor_tensor(out=acc[:], in0=g1[:, t, :],
                                             in1=g2[:, t, :], op=OP.add)
                        nc.sync.dma_start(out=out[sl(t, P), :], in_=acc[:])

            if timing_reps:
                tin = const.tile([1, 4], F32)
                nc.sync.dma_start(out=tin[:], in_=tdum[:])
                tou = const.tile([1, 4], F32)
                nc.sync.dma_start(out=tou[:], in_=out[0:1, 0:4])
                tsum = const.tile([1, 4], F32)
                nc.vector.tensor_tensor(out=tsum[:], in0=tin[:], in1=tou[:], op=OP.add)
                nc.sync.dma_start(out=outt[:], in_=tsum[:])

    nc.compile()
    return nc


def make_host_inputs(x, Wg, bg, We, be):
    """Shard + precompute host-side input arrays. Returns per-core in_maps."""
    x = np.asarray(x, np.float32)
    Wg = np.asarray(Wg, np.float32)
    bg = np.asarray(bg, np.float32)
    We = np.asarray(We, np.float32)
    be = np.asarray(be, np.float32)

    xf = x.reshape(NCORES, TOK, D)
    xhv = xf.astype(BF16)
    xrv = (xf - xhv.astype(np.float32)).astype(BF16)
    wgh = Wg.astype(BF16)
    wgr = (Wg - wgh.astype(np.float32)).astype(BF16)
    wgb = np.concatenate([wgh, wgr], axis=1)          # [D, 16]
    bgb = np.tile(bg.astype(np.float32), (P, NT))
    web = We.astype(BF16)
    beb = be.astype(BF16).reshape(1, E * D)

    idb = np.eye(P, dtype=BF16)
    idf = np.eye(P, dtype=np.float32)
    permf = np.zeros((P, P), np.float32)
    for p in range(P):
        permf[p, (p % 16) * 8 + p // 16] = 1.0
    rep16v = np.zeros((16, P), np.float32)
    for q in range(16):
        rep16v[q, q::16] = 1.0
    u128 = np.triu(np.ones((P, P), np.float32)).astype(BF16)      # k<=m
    u8x = np.triu(np.ones((E, E), np.float32), k=1).astype(np.float32)  # k<m
    ones1 = np.ones((1, P), np.float32)
    ones1b = np.ones((1, P), np.float32).astype(BF16)
    onesc = np.ones((P, 1), np.float32).astype(BF16)
    ecv = np.tile(np.arange(E, dtype=np.float32) * C, (P, NT))
    tokid = (np.arange(P, dtype=np.float32)[:, None]
             + P * np.arange(NT, dtype=np.float32)[None, :]).copy()

    shared = dict(wgb=wgb, bgb=bgb, web=web, beb=beb, idb=idb, idf=idf, permf=permf, rep16=rep16v,
                  u128=u128, u8x=u8x, ones1=ones1, ones1b=ones1b, onesc=onesc, ecv=ecv, tokid=tokid)
    in_maps = []
    for c in range(NCORES):
        m = dict(shared)
        m["xh"] = np.ascontiguousarray(xhv[c])
        m["xhT"] = np.ascontiguousarray(xhv[c].T)
        m["xrT"] = np.ascontiguousarray(xrv[c].T)
        in_maps.append(m)
    return in_maps


_NC_CACHE = None


def kernel(x, Wg, bg, We, be):
    global _NC_CACHE
    in_maps = make_host_inputs(x, Wg, bg, We, be)
    if _NC_CACHE is None:
        _NC_CACHE = build_nc()
    res = run_bass_kernel_spmd(_NC_CACHE, in_maps, list(range(NCORES)))
    outs = [np.asarray(res.results[c]["out"]).astype(np.float32) for c in range(NCORES)]
    return np.concatenate(outs, axis=0).reshape(4, 2048, D)
